# revision 1
# baseline (speedup 1.0000x reference)
"""Sliding-window GQA causal self-attention (ALiBi) Trainium2 Bass kernel.

Problem: B=2, T=4096, C=1024, H=16, HKV=4 (GQA G=4), D=64, window W=512,
fused qkv projection + sliding-window attention + output projection.

Sharding: data-parallel over (batch x T/4) -> 8 cores. Each core computes
1024 query rows of one batch plus a 512-row k/v halo. No collectives.

Per-core dataflow (fp32 data, matmuls in float32r = full-rate ~13-bit fp32):
  - x arrives host-transposed; xT streamed in 512-column time slices
  - qT/kT computed transposed (stationary wqkv chunk, streaming xT);
    v computed natural (stationary xT chunk, streaming wv)
  - scores: one K=67 matmul pair per 128-row q-block; 3 augmentation rows
    fold in the ALiBi bias (rank-2 in block-local coords) and the per-core
    left-edge -1e9 penalty
  - window mask: one strided DVE add of two 128x128 triangles; softmax with
    no max-subtraction (scores are N(0,~6.5); exp can't overflow at <13
    sigma and every row sum stays normal) - shift-invariance makes it exact
  - exp with accumulated row-sum on ACT; p scaled by 1/sum on DVE
  - p PE-transposed back into the score PSUM tile, copied into a kc-aligned
    pT slab by ACT+DVE in parallel; PV accumulates oT[d, qt] over 8 aligned
    k-chunks per half q-range
  - attnT assembled [c, t]; out = attnT.T @ wo streamed naturally
"""

import math
from contextlib import ExitStack

import numpy as np

import concourse.bass as bass
from concourse import bacc
import concourse.mybir as mybir
import concourse.tile as tile
from concourse.bass_utils import run_bass_kernel_spmd

F32 = mybir.dt.float32
F32R = mybir.dt.float32r

B, T, C = 2, 4096, 1024
H, HKV, G, D = 16, 4, 4, 64
W = 512
NCORES = 8
RT = 1024              # own query rows per core
KR = RT + W            # k/v slab rows (512 halo + 1024 own)
NQB = RT // 128        # 8 q-blocks of 128
NKC = KR // 128        # 12 k-chunks of 128
SCALE = D ** -0.5      # 0.125, exact power of two
NEG = -1e9
KCOL0 = C              # wqkv col offset of k
VCOL0 = C + HKV * D    # wqkv col offset of v


def alibi_slopes(n_head: int) -> np.ndarray:
    def slopes_power_of_2(n):
        start = 2.0 ** (-(2.0 ** (-(math.log2(n) - 3))))
        return [start * start ** i for i in range(n)]

    if float(math.log2(n_head)).is_integer():
        s = slopes_power_of_2(n_head)
    else:
        closest = 2 ** math.floor(math.log2(n_head))
        s = slopes_power_of_2(closest)
        s2 = slopes_power_of_2(2 * closest)
        s += s2[0::2][: n_head - closest]
    return np.array(s, dtype=np.float32)


def build_nc(loop: int = 1) -> bacc.Bacc:
    nc = bacc.Bacc("TRN2", target_bir_lowering=False)

    xs = nc.dram_tensor("xs", [C, KR], F32R, kind="ExternalInput")  # host-transposed
    wqkv = nc.dram_tensor("wqkv", [C, C + 2 * HKV * D], F32R, kind="ExternalInput")
    wo = nc.dram_tensor("wo", [C, C], F32R, kind="ExternalInput")
    qaug = nc.dram_tensor("qaug", [H, 3, RT], F32R, kind="ExternalInput")
    kaug = nc.dram_tensor("kaug", [3, KR], F32R, kind="ExternalInput")
    w0ab = nc.dram_tensor("w0ab", [128, 256], F32, kind="ExternalInput")
    ident = nc.dram_tensor("ident", [128, 128], F32R, kind="ExternalInput")
    out = nc.dram_tensor("out", [RT, C], F32, kind="ExternalOutput")

    with tile.TileContext(nc) as tc:
      for _rep in range(loop):
        with ExitStack() as ctx:
            persist = ctx.enter_context(tc.tile_pool(name="persist", bufs=1))

            qT = persist.tile([128, H * RT], F32R)      # [0:64] data, [64:67] aug
            kT = persist.tile([128, HKV * KR], F32R)    # [0:64] data, [64:67] aug
            vsl = persist.tile([128, NKC * HKV * D], F32R)  # v natural, kc-major
            attnT = persist.tile([128, 8 * RT], F32R)   # [c in chunk, cc*RT + t]
            id_sb = persist.tile([128, 128], F32R)
            nc.sync.dma_start(id_sb, ident[:, :])

            for h in range(H):
                nc.sync.dma_start(qT[64:67, h * RT:(h + 1) * RT], qaug[h, :, :])
            for kv in range(HKV):
                nc.sync.dma_start(kT[64:67, kv * KR:(kv + 1) * KR], kaug[:, :])

            # ---------------- Phase Q: qkv projection ----------------
            with tc.tile_pool(name="xTp", bufs=2) as xTp, \
                 tc.tile_pool(name="stg", bufs=2) as stg, \
                 tc.tile_pool(name="wqp", bufs=2) as wqp, \
                 tc.tile_pool(name="wvp", bufs=1) as wvp, \
                 tc.tile_pool(name="psQK", bufs=4, space="PSUM") as psQK, \
                 tc.tile_pool(name="psV", bufs=3, space="PSUM") as psV:

                wv = wvp.tile([128, 8 * 256], F32R)
                for cc in range(8):
                    nc.sync.dma_start(wv[:, cc * 256:(cc + 1) * 256],
                                      wqkv[cc * 128:(cc + 1) * 128, VCOL0:VCOL0 + 256])

                xTts = {}

                def build_slice(ts):
                    xTt = xTp.tile([128, 8 * 512], F32R, tag="xts")
                    for cc in range(8):
                        nc.sync.dma_start(
                            xTt[:, cc * 512:(cc + 1) * 512],
                            xs[cc * 128:(cc + 1) * 128, ts * 512:(ts + 1) * 512])
                    for tki in range(4):
                        tk = ts * 4 + tki
                        psv = psV.tile([128, 256], F32, tag="vps")
                        for cc in range(8):
                            nc.tensor.matmul(
                                psv,
                                lhsT=xTt[:, cc * 512 + tki * 128:cc * 512 + (tki + 1) * 128],
                                rhs=wv[:, cc * 256:(cc + 1) * 256],
                                start=(cc == 0), stop=(cc == 7))
                        nc.any.tensor_copy(vsl[:, tk * 256:(tk + 1) * 256], psv)
                    return xTt

                def qk_slab(fc2, ts_list):
                    # fc2 0..3: q feature pairs; fc2 4: k features (both kv pairs)
                    fcol = fc2 * 256 if fc2 < 4 else KCOL0
                    wq = wqp.tile([128, 8 * 256], F32R, tag="wqf")
                    for cc in range(8):
                        nc.sync.dma_start(
                            wq[:, cc * 256:(cc + 1) * 256],
                            wqkv[cc * 128:(cc + 1) * 128, fcol:fcol + 256])
                    for ts in ts_list:
                        for fi in range(2):
                            fc = fc2 * 2 + fi
                            ps = psQK.tile([128, 512], F32, tag="qkps")
                            for cc in range(8):
                                nc.tensor.matmul(
                                    ps,
                                    lhsT=wq[:, cc * 256 + fi * 128:cc * 256 + (fi + 1) * 128],
                                    rhs=xTts[ts][:, cc * 512:(cc + 1) * 512],
                                    start=(cc == 0), stop=(cc == 7))
                            st = stg.tile([128, 512], F32R, tag="stg")
                            nc.any.tensor_copy(st[64:128, :], ps[64:128, :])
                            if fc2 < 4:
                                h0, h1 = 2 * fc, 2 * fc + 1
                                toff = (ts - 1) * 512
                                nc.any.tensor_copy(
                                    qT[0:64, h0 * RT + toff:h0 * RT + toff + 512],
                                    ps[0:64, :])
                                nc.sync.dma_start(
                                    qT[0:64, h1 * RT + toff:h1 * RT + toff + 512],
                                    st[64:128, :])
                            else:
                                kv0, kv1 = 2 * fi, 2 * fi + 1
                                toff = ts * 512
                                nc.any.tensor_copy(
                                    kT[0:64, kv0 * KR + toff:kv0 * KR + toff + 512],
                                    ps[0:64, :])
                                nc.sync.dma_start(
                                    kT[0:64, kv1 * KR + toff:kv1 * KR + toff + 512],
                                    st[64:128, :])

                xTts[0] = build_slice(0)
                qk_slab(4, [0])
                xTts[1] = build_slice(1)
                xTts[2] = build_slice(2)
                qk_slab(4, [1, 2])
                for fc2 in range(4):
                    qk_slab(fc2, [1, 2])

            # -------- wo prefetch (overlaps attention) --------
            wop = ctx.enter_context(tc.tile_pool(name="wop", bufs=1))
            wo_sb = wop.tile([128, 8 * 1024], F32R)
            for cc in range(8):
                nc.sync.dma_start(wo_sb[:, cc * 1024:(cc + 1) * 1024],
                                  wo[cc * 128:(cc + 1) * 128, :])

            # ---------------- Phase A: attention ----------------
            with tc.tile_pool(name="phA", bufs=2) as pha, \
                 tc.tile_pool(name="mk", bufs=1) as mk, \
                 tc.tile_pool(name="stO", bufs=1) as stO, \
                 tc.tile_pool(name="pTp", bufs=1) as pTp, \
                 tc.tile_pool(name="psS", bufs=3, space="PSUM") as psS, \
                 tc.tile_pool(name="psO", bufs=2, space="PSUM") as psO:

                w0ab_sb = mk.tile([128, 256], F32)
                nc.sync.dma_start(w0ab_sb, w0ab[:, :])

                pTa = pTp.tile([128, 8 * 512], F32R)  # [kt, slot*512 + qt']
                pTb = pTp.tile([128, 8 * 512], F32R)
                pTs = [pTa, pTb]
                zs = mk.tile([128, 512], F32)
                nc.vector.memset(zs, 0.0)
                for j in range(8):
                    nc.any.tensor_copy(pTa[:, j * 512:(j + 1) * 512], zs)
                    nc.any.tensor_copy(pTb[:, j * 512:(j + 1) * 512], zs)

                for kv in range(HKV):
                    for gp in range(2):
                        for half in range(2):
                            for qbp_gi in range(8):
                                qbp, gi = qbp_gi // 2, qbp_gi % 2
                                g = gp * 2 + gi
                                h = kv * G + g
                                pT = pTs[gi]
                                qb = half * 4 + qbp
                                stile = psS.tile([128, 640], F32, tag="sc")
                                qstat = qT[0:67, h * RT + qb * 128:h * RT + (qb + 1) * 128]
                                kbase = kv * KR + qb * 128
                                nc.tensor.matmul(stile[:, 0:512], lhsT=qstat,
                                                 rhs=kT[0:67, kbase:kbase + 512],
                                                 start=True, stop=True)
                                nc.tensor.matmul(stile[:, 512:640], lhsT=qstat,
                                                 rhs=kT[0:67, kbase + 512:kbase + 640],
                                                 start=True, stop=True)
                                sm = stile[:, 0:128]
                                mreg = bass.AP(tensor=sm.tensor, offset=sm.offset,
                                               ap=[list(sm.ap[0]), [512, 2], [1, 128]])
                                nc.vector.tensor_add(
                                    mreg, mreg,
                                    w0ab_sb.rearrange("p (a b) -> p a b", b=128))
                                # No max-subtraction: scores are N(0, ~6.5); exp
                                # overflow needs ~13 sigma; every row sum stays in
                                # normal fp32 range. Softmax shift-invariance keeps
                                # this exact w.r.t. the reference.
                                p = pha.tile([128, 640], F32R, tag="p")
                                ssum = pha.tile([128, 1], F32, tag="ssum")
                                nc.scalar.activation(p, stile,
                                                     mybir.ActivationFunctionType.Exp,
                                                     bias=0.0, accum_out=ssum)
                                rs = pha.tile([128, 1], F32, tag="rs")
                                nc.vector.reciprocal(rs, ssum)
                                nc.vector.tensor_scalar_mul(p, p, rs)
                                sb16 = stile.bitcast(F32R)
                                for ck in range(5):
                                    nc.tensor.transpose(
                                        sb16[:, ck * 128:(ck + 1) * 128],
                                        p[:, ck * 128:(ck + 1) * 128], id_sb)
                                pb = pT[:, qbp * 640:qbp * 640 + 128]
                                dstA = bass.AP(tensor=pb.tensor, offset=pb.offset,
                                               ap=[list(pb.ap[0]), [512, 2], [1, 128]])
                                pb3 = pT[:, qbp * 640 + 1024:qbp * 640 + 1024 + 128]
                                dstB = bass.AP(tensor=pb3.tensor, offset=pb3.offset,
                                               ap=[list(pb3.ap[0]), [512, 3], [1, 128]])
                                nc.scalar.copy(dstA, sb16[:, 0:256].rearrange(
                                    "p (a b) -> p a b", b=128))
                                nc.vector.tensor_copy(dstB, sb16[:, 256:640].rearrange(
                                    "p (a b) -> p a b", b=128))
                            # PV for both heads of the pair
                            for gi in range(2):
                                g = gp * 2 + gi
                                h = kv * G + g
                                oT = psO.tile([64, 512], F32, tag="oT")
                                for s in range(8):
                                    kc = half * 4 + s
                                    nc.tensor.matmul(
                                        oT,
                                        lhsT=vsl[:, kc * 256 + kv * 64:kc * 256 + (kv + 1) * 64],
                                        rhs=pTs[gi][:, s * 512:(s + 1) * 512],
                                        start=(s == 0), stop=(s == 7))
                                cc = h // 2
                                cb = cc * RT + half * 512
                                if h % 2 == 0:
                                    nc.any.tensor_copy(attnT[0:64, cb:cb + 512], oT)
                                else:
                                    so = stO.tile([64, 512], F32R, tag="so")
                                    nc.any.tensor_copy(so, oT)
                                    nc.sync.dma_start(attnT[64:128, cb:cb + 512], so)

            # ---------------- Phase O: output projection ----------------
            with tc.tile_pool(name="phO", bufs=3) as pho, \
                 tc.tile_pool(name="psF", bufs=3, space="PSUM") as psF:
                for tk in range(8):
                    for ec in range(2):
                        ps = psF.tile([128, 512], F32, tag="fps")
                        for cc in range(8):
                            nc.tensor.matmul(
                                ps,
                                lhsT=attnT[:, cc * RT + tk * 128:cc * RT + (tk + 1) * 128],
                                rhs=wo_sb[:, cc * 1024 + ec * 512:cc * 1024 + ec * 512 + 512],
                                start=(cc == 0), stop=(cc == 7))
                        ob = pho.tile([128, 512], F32, tag="ob")
                        nc.any.tensor_copy(ob, ps)
                        nc.sync.dma_start(
                            out[tk * 128:(tk + 1) * 128, ec * 512:(ec + 1) * 512], ob)

    nc.compile()
    return nc


_NC = None


def _host_inputs(x, wqkv, wo):
    slopes = alibi_slopes(H)  # head h = kv*G + g matches slopes.reshape(HKV, G)

    wqkv_s = np.array(wqkv, dtype=np.float32, copy=True)
    wqkv_s[:, :C] *= SCALE  # exact power-of-two fold of the score scale into wq

    j = np.arange(RT, dtype=np.float32)
    qaug = np.empty((H, 3, RT), dtype=np.float32)
    for h in range(H):
        qaug[h, 0] = -slopes[h] * (j + 512.0)
        qaug[h, 1] = slopes[h]
        qaug[h, 2] = 1.0

    i = np.arange(KR, dtype=np.float32)
    kaug_base = np.empty((3, KR), dtype=np.float32)
    kaug_base[0] = 1.0
    kaug_base[1] = i
    kaug_base[2] = 0.0

    r = np.arange(128)[:, None]
    l = np.arange(128)[None, :]
    w0a = np.where(l <= r, np.float32(NEG), np.float32(0.0)).astype(np.float32)
    w0b = np.where(l > r, np.float32(NEG), np.float32(0.0)).astype(np.float32)
    w0ab = np.concatenate([w0a, w0b], axis=1)
    ident = np.eye(128, dtype=np.float32)

    in_maps = []
    for core in range(NCORES):
        b, qq = core // 4, core % 4
        t0 = qq * RT
        xsl = np.zeros((KR, C), dtype=np.float32)
        lo = t0 - W
        if lo < 0:
            xsl[-lo:, :] = x[b, 0:t0 + RT, :]
        else:
            xsl[:, :] = x[b, lo:t0 + RT, :]
        xsl = np.ascontiguousarray(xsl.T)
        kaug = kaug_base.copy()
        if lo < 0:
            kaug[2, :W] = NEG  # left-edge penalty kills padded keys
        in_maps.append(dict(xs=xsl, wqkv=wqkv_s, wo=np.asarray(wo, dtype=np.float32),
                            qaug=qaug, kaug=kaug, w0ab=w0ab, ident=ident))
    return in_maps


def kernel(x, wqkv, wo):
    global _NC
    if _NC is None:
        _NC = build_nc()
    in_maps = _host_inputs(np.asarray(x), np.asarray(wqkv), np.asarray(wo))
    res = run_bass_kernel_spmd(_NC, in_maps, list(range(NCORES)))
    full = np.empty((B, T, C), dtype=np.float32)
    for core in range(NCORES):
        b, qq = core // 4, core % 4
        full[b, qq * RT:(qq + 1) * RT, :] = res.results[core]["out"]
    return full



# revision 7
# speedup vs baseline: 1.3169x; 1.3169x over previous
"""Sliding-window GQA causal self-attention (ALiBi) Trainium2 Bass kernel.

Problem: B=2, T=4096, C=1024, H=16, HKV=4 (GQA G=4), D=64, window W=512,
fused qkv projection + sliding-window attention + output projection.

Sharding: data-parallel over (batch x T/4) -> 8 cores. Each core computes
1024 query rows of one batch plus a 512-row k/v halo. No collectives.

Per-core dataflow (matmuls in float32r except PV/out-proj in bf16):
  - x arrives host-transposed; xT streamed in 256-column time slices
  - qT computed transposed into a packed layout [kv][qb*512 + g*128 + q]
    so one N=512 score matmul covers all 4 GQA heads of a kv group;
    kT computed transposed per kv head; v natural (bf16) with a ones
    column appended per (chunk, kv) block
  - scores computed TRANSPOSED directly: sT[keys, q] = kT_aug^T @ qT_aug
    per 128-key chunk (kT chunk stationary, K=67; 3 augmentation rows
    fold the ALiBi bias and left-edge -1e9 penalty) - no PE transposes
  - window mask: triangle adds on the two extreme chunks only (DVE)
  - softmax without max-subtraction (scores are N(0,~6.5); exp cannot
    overflow at <13 sigma) - shift-invariance keeps it exact
  - PV accumulates oT[65, 512] over exactly 5 key chunks; the v ones
    column makes row 64 the softmax denominator for free
  - normalization AFTER PV: reciprocal of one [1,512] row, broadcast
    across 64 partitions via a K=1 matmul, one [64,512] multiply into
    bf16 attnT (even g direct, odd g staged through an SBUF DMA for the
    partition shift); deferred one kv step so the PE never stalls on it
  - out = attnT.T @ wo (bf16) per q-block, interleaved with attention
"""

import math
from contextlib import ExitStack

import numpy as np
import ml_dtypes

import concourse.bass as bass
from concourse import bacc
import concourse.mybir as mybir
import concourse.tile as tile
from concourse.bass_utils import run_bass_kernel_spmd

F32 = mybir.dt.float32
F32R = mybir.dt.float32r
BF16 = mybir.dt.bfloat16

B, T, C = 2, 4096, 1024
H, HKV, G, D = 16, 4, 4, 64
W = 512
NCORES = 8
RT = 1024              # own query rows per core
KR = RT + W            # k/v slab rows (512 halo + 1024 own)
NQB = RT // 128        # 8 q-blocks of 128
NKC = KR // 128        # 12 k-chunks of 128
SCALE = D ** -0.5      # 0.125, exact power of two
NEG = -1e9
KCOL0 = C              # wqkv col offset of k
VCOL0 = C + HKV * D    # wqkv col offset of v
VW = 65                # v block width: 64 features + ones column


def alibi_slopes(n_head: int) -> np.ndarray:
    def slopes_power_of_2(n):
        start = 2.0 ** (-(2.0 ** (-(math.log2(n) - 3))))
        return [start * start ** i for i in range(n)]

    if float(math.log2(n_head)).is_integer():
        s = slopes_power_of_2(n_head)
    else:
        closest = 2 ** math.floor(math.log2(n_head))
        s = slopes_power_of_2(closest)
        s2 = slopes_power_of_2(2 * closest)
        s += s2[0::2][: n_head - closest]
    return np.array(s, dtype=np.float32)


def build_nc(loop: int = 1) -> bacc.Bacc:
    nc = bacc.Bacc("TRN2", target_bir_lowering=False)

    xs = nc.dram_tensor("xs", [C, KR], F32R, kind="ExternalInput")  # host-transposed
    wqkv = nc.dram_tensor("wqkv", [C, C + 2 * HKV * D], F32R, kind="ExternalInput")
    wob = nc.dram_tensor("wob", [C, C], BF16, kind="ExternalInput")
    qaug = nc.dram_tensor("qaug", [HKV, 3, G * RT], F32R, kind="ExternalInput")
    kaug = nc.dram_tensor("kaug", [3, KR], F32R, kind="ExternalInput")
    w0q4 = nc.dram_tensor("w0q4", [128, 512], F32, kind="ExternalInput")
    w4q4 = nc.dram_tensor("w4q4", [128, 512], F32, kind="ExternalInput")
    out = nc.dram_tensor("out", [RT, C], F32, kind="ExternalOutput")

    Exp = mybir.ActivationFunctionType.Exp

    with tile.TileContext(nc) as tc:
      for _rep in range(loop):
        with ExitStack() as ctx:
            persist = ctx.enter_context(tc.tile_pool(name="persist", bufs=1))

            # packed qT per kv: rows 0:64 data, 64:67 aug; col = qb*512+g*128+q
            qTs = [persist.tile([128, NQB * 512], F32R, name=f"qT{kv}")
                   for kv in range(HKV)]
            kTs = [persist.tile([128, KR], F32R, name=f"kT{kv}")
                   for kv in range(HKV)]
            # v natural bf16, kc-major: [t-in-chunk, kc*(4*65) + kv*65 + (d|ones)]
            vsl = persist.tile([128, NKC * HKV * VW], BF16)
            wo_sb = persist.tile([128, 8 * 1024], BF16)
            w0_sb = persist.tile([128, 512], F32)
            w4_sb = persist.tile([128, 512], F32)
            ones65 = persist.tile([65, 64], F32R)

            nc.gpsimd.memset(ones65[64:65, :].bitcast(F32), 1.0)
            # ones column of every (kc, kv) v block
            vones = bass.AP(tensor=vsl.tensor, offset=vsl.offset + 64,
                            ap=[list(vsl.ap[0]), [HKV * VW, NKC], [VW, HKV]])
            nc.vector.memset(vones, 1.0)

            nc.sync.dma_start(w0_sb, w0q4[:, :])
            nc.sync.dma_start(w4_sb, w4q4[:, :])
            for kv in range(HKV):
                nc.sync.dma_start(qTs[kv][64:67, :], qaug[kv, :, :])
                nc.sync.dma_start(kTs[kv][64:67, :], kaug[:, :])

            xTp = ctx.enter_context(tc.tile_pool(name="xTp", bufs=2))
            wqp = ctx.enter_context(tc.tile_pool(name="wqp", bufs=1))
            stp = ctx.enter_context(tc.tile_pool(name="stp", bufs=3))
            pTp = ctx.enter_context(tc.tile_pool(name="pTp", bufs=6))
            atp = ctx.enter_context(tc.tile_pool(name="atp", bufs=2))
            obp = ctx.enter_context(tc.tile_pool(name="obp", bufs=2))
            sgp = ctx.enter_context(tc.tile_pool(name="sgp", bufs=3))
            rsp = ctx.enter_context(tc.tile_pool(name="rsp", bufs=2))
            bcp = ctx.enter_context(tc.tile_pool(name="bcp", bufs=2))
            psA = ctx.enter_context(tc.tile_pool(name="psA", bufs=2, space="PSUM"))
            psS = ctx.enter_context(tc.tile_pool(name="psS", bufs=3, space="PSUM"))
            psO = ctx.enter_context(tc.tile_pool(name="psO", bufs=2, space="PSUM"))
            psB = ctx.enter_context(tc.tile_pool(name="psB", bufs=1, space="PSUM"))

            # stationary weights: wk/wv/wq resident for the whole kernel
            wk = wqp.tile([128, 8 * 256], F32R)
            wv = wqp.tile([128, 8 * 256], F32R)
            wqs = [wqp.tile([128, 8 * 256], F32R, name=f"wq{kv}")
                   for kv in range(HKV)]
            for cc in range(8):
                nc.sync.dma_start(wk[:, cc * 256:(cc + 1) * 256],
                                  wqkv[cc * 128:(cc + 1) * 128, KCOL0:KCOL0 + 256])
                nc.sync.dma_start(wv[:, cc * 256:(cc + 1) * 256],
                                  wqkv[cc * 128:(cc + 1) * 128, VCOL0:VCOL0 + 256])
                for kv in range(HKV):
                    nc.sync.dma_start(
                        wqs[kv][:, cc * 256:(cc + 1) * 256],
                        wqkv[cc * 128:(cc + 1) * 128, kv * 256:kv * 256 + 256])

            def load_slice(ts):
                xTt = xTp.tile([128, 8 * 256], F32R, tag="xts")
                for cc in range(8):
                    nc.sync.dma_start(
                        xTt[:, cc * 256:(cc + 1) * 256],
                        xs[cc * 128:(cc + 1) * 128, ts * 256:(ts + 1) * 256])
                return xTt

            def proj_slice(ts, xTt):
                t0 = ts * 256
                # k projection: fi=0 -> (kv0,kv1), fi=1 -> (kv2,kv3)
                for fi in range(2):
                    pst = psA.tile([128, 512], F32, tag="ps")
                    ps = pst[:, 0:256]
                    for cc in range(8):
                        nc.tensor.matmul(
                            ps,
                            lhsT=wk[:, cc * 256 + fi * 128:cc * 256 + (fi + 1) * 128],
                            rhs=xTt[:, cc * 256:(cc + 1) * 256],
                            start=(cc == 0), stop=(cc == 7))
                    kv0, kv1 = 2 * fi, 2 * fi + 1
                    nc.scalar.copy(kTs[kv0][0:64, t0:t0 + 256], ps[0:64, :])
                    st = stp.tile([128, 256], F32R, tag="st")
                    nc.vector.tensor_copy(st[64:128, :], ps[64:128, :])
                    nc.sync.dma_start(kTs[kv1][0:64, t0:t0 + 256], st[64:128, :])
                # v projection: two 128-t chunks per slice
                for tki in range(2):
                    kc = ts * 2 + tki
                    psvt = psA.tile([128, 512], F32, tag="ps")
                    psv = psvt[:, 0:256]
                    for cc in range(8):
                        nc.tensor.matmul(
                            psv,
                            lhsT=xTt[:, cc * 256 + tki * 128:cc * 256 + (tki + 1) * 128],
                            rhs=wv[:, cc * 256:(cc + 1) * 256],
                            start=(cc == 0), stop=(cc == 7))
                    vdst = bass.AP(tensor=vsl.tensor,
                                   offset=vsl.offset + kc * HKV * VW,
                                   ap=[list(vsl.ap[0]), [VW, HKV], [1, 64]])
                    nc.scalar.copy(vdst, psv.rearrange("p (a b) -> p a b", b=64))
                # q projection (own rows only)
                if ts >= 2:
                    toff = t0 - 512
                    qb0 = toff // 128
                    for kv in range(HKV):
                        for fi in range(2):
                            pst = psA.tile([128, 512], F32, tag="ps")
                            ps = pst[:, 0:256]
                            for cc in range(8):
                                nc.tensor.matmul(
                                    ps,
                                    lhsT=wqs[kv][:, cc * 256 + fi * 128:
                                                 cc * 256 + (fi + 1) * 128],
                                    rhs=xTt[:, cc * 256:(cc + 1) * 256],
                                    start=(cc == 0), stop=(cc == 7))
                            ge, go = 2 * fi, 2 * fi + 1
                            qd = qTs[kv]
                            dste = bass.AP(
                                tensor=qd.tensor,
                                offset=qd.offset + qb0 * 512 + ge * 128,
                                ap=[[qd.ap[0][0], 64], [512, 2], [1, 128]])
                            nc.scalar.copy(
                                dste, ps[0:64, :].rearrange("p (a b) -> p a b", b=128))
                            st = stp.tile([128, 256], F32R, tag="st")
                            nc.vector.tensor_copy(st[64:128, :], ps[64:128, :])
                            dsto = bass.AP(
                                tensor=qd.tensor,
                                offset=qd.offset + qb0 * 512 + go * 128,
                                ap=[[qd.ap[0][0], 64], [512, 2], [1, 128]])
                            nc.sync.dma_start(dsto, st[64:128, :].rearrange(
                                "p (a b) -> p a b", b=128))

            def pair(src, off):
                s64 = src[0:64, :]
                return bass.AP(tensor=s64.tensor, offset=s64.offset + off,
                               ap=[list(s64.ap[0]), [256, 2], [1, 128]])

            def emit_scale(at, po, rs, kv):
                # broadcast 1/s across 64 partitions with a K=1 matmul
                pb = psB.tile([64, 512], F32, tag="bc")
                nc.tensor.matmul(pb, lhsT=ones65[64:65, :], rhs=rs[64:65, :],
                                 start=True, stop=True)
                bcs = bcp.tile([64, 512], F32R, tag="bcs")
                nc.scalar.copy(bcs, pb)
                # even g -> attnT rows 0:64 directly; odd g staged via DMA
                nc.vector.tensor_mul(at[0:64, kv * 256:kv * 256 + 256],
                                     pair(po, 0), pair(bcs, 0))
                sg = sgp.tile([64, 256], BF16, tag="sg")
                nc.vector.tensor_mul(sg, pair(po, 128), pair(bcs, 128))
                nc.sync.dma_start(at[64:128, kv * 256:kv * 256 + 256], sg)

            pend = {}

            def flush_pending():
                # deferred (qb, kv=3) scale of the previous q-block, then its
                # output projection
                if not pend:
                    return
                at, po, rs, qb = pend["at"], pend["po"], pend["rs"], pend["qb"]
                emit_scale(at, po, rs, 3)
                for ec in range(2):
                    pf = psA.tile([128, 512], F32, tag="ps")
                    for cc in range(8):
                        nc.tensor.matmul(
                            pf,
                            lhsT=at[:, cc * 128:(cc + 1) * 128],
                            rhs=wo_sb[:, cc * 1024 + ec * 512:
                                      cc * 1024 + ec * 512 + 512],
                            start=(cc == 0), stop=(cc == 7))
                    ob = obp.tile([128, 512], F32, tag="ob")
                    nc.scalar.copy(ob, pf)
                    nc.sync.dma_start(
                        out[qb * 128:(qb + 1) * 128, ec * 512:(ec + 1) * 512], ob)
                pend.clear()

            def attn_block(qb):
                at = atp.tile([128, 8 * 128], BF16, tag="at")
                blk = {}
                for kv in range(HKV):
                    pcs = []
                    for j in range(5):
                        ck = qb + j
                        ps = psS.tile([128, 512], F32, tag="sc")
                        nc.tensor.matmul(
                            ps,
                            lhsT=kTs[kv][0:67, ck * 128:(ck + 1) * 128],
                            rhs=qTs[kv][0:67, qb * 512:(qb + 1) * 512],
                            start=True, stop=True)
                        if j == 0:
                            nc.vector.tensor_add(ps, ps, w0_sb)
                        elif j == 4:
                            nc.vector.tensor_add(ps, ps, w4_sb)
                        pc = pTp.tile([128, 512], BF16, tag="pc")
                        nc.scalar.activation(pc, ps, Exp, bias=0.0)
                        pcs.append(pc)
                    # deferred scaling keeps the PE from stalling on recip
                    if kv == 0:
                        flush_pending()
                    else:
                        ppo, prs = blk[kv - 1]
                        emit_scale(at, ppo, prs, kv - 1)
                    po = psO.tile([65, 512], F32, tag="ot")
                    for j in range(5):
                        base = (qb + j) * HKV * VW + kv * VW
                        nc.tensor.matmul(po, lhsT=vsl[:, base:base + VW],
                                         rhs=pcs[j], start=(j == 0), stop=(j == 4))
                    rs = rsp.tile([65, 512], F32R, tag="rs")
                    with nc.allow_low_precision(reason="f32r holds full fp32 bits"):
                        nc.vector.reciprocal(rs[64:65, :], po[64:65, :])
                    blk[kv] = (po, rs)
                pend.update(at=at, po=blk[3][0], rs=blk[3][1], qb=qb)

            # ---------------- schedule ----------------
            xts = [load_slice(0), load_slice(1), load_slice(2)]
            proj_slice(0, xts[0])
            proj_slice(1, xts[1])
            proj_slice(2, xts[2])
            for cc in range(8):
                nc.sync.dma_start(wo_sb[:, cc * 1024:(cc + 1) * 1024],
                                  wob[cc * 128:(cc + 1) * 128, :])
            for ts in range(3, 6):
                xts.append(load_slice(ts))
                attn_block(2 * ts - 6)
                attn_block(2 * ts - 5)
                proj_slice(ts, xts[ts])
            attn_block(6)
            attn_block(7)
            flush_pending()

    nc.compile()
    return nc


_NC = None


def _host_inputs(x, wqkv, wo):
    slopes = alibi_slopes(H)  # head h = kv*G + g matches slopes.reshape(HKV, G)

    wqkv_s = np.array(wqkv, dtype=np.float32, copy=True)
    wqkv_s[:, :C] *= SCALE  # exact power-of-two fold of the score scale into wq

    # packed q augmentation: col = qb*512 + g*128 + q, t = qb*128 + q
    qaug = np.empty((HKV, 3, G * RT), dtype=np.float32)
    cols = np.arange(G * RT)
    col_t = (cols // 512) * 128 + (cols % 128)
    col_g = (cols % 512) // 128
    for kv in range(HKV):
        sl = slopes[kv * G + col_g]
        qaug[kv, 0] = -sl * (col_t + 512.0)
        qaug[kv, 1] = sl
        qaug[kv, 2] = 1.0

    i = np.arange(KR, dtype=np.float32)
    kaug_base = np.empty((3, KR), dtype=np.float32)
    kaug_base[0] = 1.0
    kaug_base[1] = i
    kaug_base[2] = 0.0

    # transposed-score window masks on the extreme chunks, tiled for 4 g:
    # chunk j=0: valid q < r  -> mask q >= r; chunk j=4: valid q >= r
    r = np.arange(128)[:, None]
    q = np.arange(128)[None, :]
    w0 = np.where(q >= r, np.float32(NEG), np.float32(0.0)).astype(np.float32)
    w4 = np.where(q < r, np.float32(NEG), np.float32(0.0)).astype(np.float32)
    w0q4 = np.ascontiguousarray(np.tile(w0, (1, 4)))
    w4q4 = np.ascontiguousarray(np.tile(w4, (1, 4)))

    wob = np.asarray(wo, dtype=np.float32).astype(ml_dtypes.bfloat16)

    in_maps = []
    for core in range(NCORES):
        b, qq = core // 4, core % 4
        t0 = qq * RT
        xsl = np.zeros((KR, C), dtype=np.float32)
        lo = t0 - W
        if lo < 0:
            xsl[-lo:, :] = x[b, 0:t0 + RT, :]
        else:
            xsl[:, :] = x[b, lo:t0 + RT, :]
        xsl = np.ascontiguousarray(xsl.T)
        kaug = kaug_base.copy()
        if lo < 0:
            kaug[2, :W] = NEG  # left-edge penalty kills padded keys
        in_maps.append(dict(xs=xsl, wqkv=wqkv_s, wob=wob,
                            qaug=qaug, kaug=kaug, w0q4=w0q4, w4q4=w4q4))
    return in_maps


def kernel(x, wqkv, wo):
    global _NC
    if _NC is None:
        _NC = build_nc()
    in_maps = _host_inputs(np.asarray(x), np.asarray(wqkv), np.asarray(wo))
    res = run_bass_kernel_spmd(_NC, in_maps, list(range(NCORES)))
    full = np.empty((B, T, C), dtype=np.float32)
    for core in range(NCORES):
        b, qq = core // 4, core % 4
        full[b, qq * RT:(qq + 1) * RT, :] = res.results[core]["out"]
    return full
